# revision 1
# baseline (speedup 1.0000x reference)
"""Trainium2 Bass kernel for dilated local attention (nn_DilateAttention).

Problem: x [8, 64, 64, 256] f32, W_qkv [768, 256] f32.
  qkv = x @ W_qkv.T; per pixel, per head (8 heads x 32 dim): attention over
  the 9 dilated (3x3, dilation 3) spatial neighbors with zero padding.

Strategy: data-parallel over batch across 8 cores (1 image per core).
On-chip layout is transposed [c, m] (m = flat pixel index) so each of the
9 neighbor offsets delta = 64*dr + dc is a free-dim slice of a zero-border-
padded k/v buffer. PE does the qkv projection (float32r), the per-head
score reduction (bf16 product tile as stationary operand against a
block-ones moving operand, giving scores in [m, head*9+kk] layout), the
attn broadcast over head dims, and the weighted-sum accumulation (identity
lhsT, PSUM accumulate) in bf16. DVE does elementwise muls and the softmax
reductions, ACT does exp (with the 1/sqrt(dph) scale folded in) and PSUM
evacuations. Column-wrap reads are fixed with a 0/1 mask plus a count
correction on the softmax denominator (reference zero-pads keys, so invalid
slots contribute exp(0)=1 to the denominator and 0 to the numerator).
"""

import sys

sys.path.insert(0, "/opt/trn_rl_repo")

import numpy as np
import ml_dtypes
from contextlib import ExitStack

import concourse.bass as bass
import concourse.bacc as bacc
import concourse.tile as tile
from concourse import mybir
from concourse.bass_utils import run_bass_kernel_spmd

B, H, W, C = 8, 64, 64, 256
NH, DPH, K2 = 8, 32, 9
N = H * W          # 4096 pixels
PAD = 256          # zero border on each side of k/v (covers |delta| <= 195)
MCH = 512          # pixels per m-chunk
NCH = N // MCH     # 8 chunks
NSUB = N // 128    # 32 m-subchunks (scores/softmax granularity)
SCALE = DPH ** -0.5
F32 = mybir.dt.float32
F32R = mybir.dt.float32r
BF16 = mybir.dt.bfloat16
NPBF16 = ml_dtypes.bfloat16

DELTAS = [64 * (3 * i - 3) + (3 * j - 3) for i in range(3) for j in range(3)]


def _host_consts():
    ident = np.eye(128, dtype=np.float32)
    identb = np.eye(128, dtype=NPBF16)
    # score reduce (moving operand): ones_s[p, nn] = 1 iff p//32 == nn
    # (the 1/sqrt(dph) scale is applied inside the exp activation)
    ones_s = np.zeros((128, 4), NPBF16)
    for p in range(128):
        ones_s[p, p // 32] = 1.0
    # attn broadcast: B[p, j, kk, q] = 1 iff p == (4j + q//32)*9 + kk
    bkk = np.zeros((72, 2, 9, 128), NPBF16)
    for jj in range(2):
        for kk in range(9):
            for q in range(128):
                bkk[(4 * jj + q // 32) * 9 + kk, jj, kk, q] = 1.0
    bkk = bkk.reshape(72, 2 * 9 * 128)
    # r broadcast over kk within [72, m] layout: b9[h, p] = 1 iff p//9 == h
    b9 = np.zeros((8, 72), np.float32)
    for p in range(72):
        b9[p // 9, p] = 1.0
    # column-validity mask in [m-sub, f=h*9+kk] layout, plus invalid counts
    maskT = np.zeros((128, NSUB, 72), NPBF16)
    cntT = np.zeros((128, NSUB), np.float32)
    for ms in range(NSUB):
        m = ms * 128 + np.arange(128)
        jm = m % 64
        for kk in range(9):
            dc = 3 * (kk % 3) - 3
            valid = (((jm + dc) >= 0) & ((jm + dc) < 64)).astype(np.float32)
            for h in range(8):
                maskT[:, ms, h * 9 + kk] = valid
            cntT[:, ms] += 1.0 - valid
    maskT = maskT.reshape(128, NSUB * 72)
    return ident, identb, ones_s, bkk, b9, maskT, cntT


def build_nc() -> bass.Bass:
    nc = bacc.Bacc()
    x_d = nc.declare_dram_parameter("x", [N, C], F32, isOutput=False)
    w_d = nc.declare_dram_parameter("w", [3 * C, C], F32, isOutput=False)
    ident_d = nc.declare_dram_parameter("ident", [128, 128], F32, isOutput=False)
    identb_d = nc.declare_dram_parameter("identb", [128, 128], BF16, isOutput=False)
    ones_s_d = nc.declare_dram_parameter("ones_s", [128, 4], BF16, isOutput=False)
    bkk_d = nc.declare_dram_parameter("bkk", [72, 2 * 9 * 128], BF16, isOutput=False)
    b9_d = nc.declare_dram_parameter("b9", [8, 72], F32, isOutput=False)
    maskT_d = nc.declare_dram_parameter("maskT", [128, NSUB * 72], BF16, isOutput=False)
    cntT_d = nc.declare_dram_parameter("cntT", [128, NSUB], F32, isOutput=False)
    out_d = nc.declare_dram_parameter("out", [N, C], F32, isOutput=True)

    with tile.TileContext(nc) as tc, ExitStack() as ctx:
        # ---- persistent SBUF pools ----
        singles = ctx.enter_context(tc.tile_pool(name="singles", bufs=1))
        qkv_pool = ctx.enter_context(tc.tile_pool(name="qkv", bufs=1))

        ident = singles.tile([128, 128], F32)
        nc.gpsimd.dma_start(out=ident, in_=ident_d[:, :])
        identb = singles.tile([128, 128], BF16)
        nc.gpsimd.dma_start(out=identb, in_=identb_d[:, :])
        ones_s = singles.tile([128, 4], BF16)
        nc.gpsimd.dma_start(out=ones_s, in_=ones_s_d[:, :])
        bkk = singles.tile([72, 2, 9, 128], BF16)
        nc.gpsimd.dma_start(
            out=bkk, in_=bkk_d[:, :].rearrange("p (j k q) -> p j k q", j=2, k=9))
        b9t = singles.tile([8, 72], F32)
        nc.gpsimd.dma_start(out=b9t, in_=b9_d[:, :])
        maskT = singles.tile([128, NSUB, 72], BF16)
        nc.gpsimd.dma_start(
            out=maskT, in_=maskT_d[:, :].rearrange("p (s f) -> p s f", f=72))
        cntT = singles.tile([128, NSUB], F32)
        nc.gpsimd.dma_start(out=cntT, in_=cntT_d[:, :])

        # q/k/v in transposed [c, m] bf16 layout; k/v have zero borders of PAD
        qT = [qkv_pool.tile([128, N], BF16, name=f"qT{j}") for j in range(2)]
        kT = [qkv_pool.tile([128, N + 2 * PAD], BF16, name=f"kT{j}") for j in range(2)]
        vT = [qkv_pool.tile([128, N + 2 * PAD], BF16, name=f"vT{j}") for j in range(2)]
        for j in range(2):
            nc.gpsimd.memset(kT[j][:, 0:PAD], 0.0)
            nc.gpsimd.memset(kT[j][:, PAD + N:], 0.0)
            nc.gpsimd.memset(vT[j][:, 0:PAD], 0.0)
            nc.gpsimd.memset(vT[j][:, PAD + N:], 0.0)

        # ---- P0+P1: W^T tiles and x^T via PE transpose ----
        xt_pool = tc.alloc_tile_pool(name="xt_pool", bufs=1)
        with tc.tile_pool(name="trans_sb", bufs=4) as tsb, \
             tc.tile_pool(name="trans_ps", bufs=2, space="PSUM") as tps:
            wlhsT = [singles.tile([128, 6, 128], F32R, name=f"wlhsT{j}") for j in range(2)]
            for ot in range(6):
                w_rows = tsb.tile([128, 256], F32, name="w_rows")
                nc.gpsimd.dma_start(out=w_rows, in_=w_d[ot * 128:(ot + 1) * 128, :])
                for j in range(2):
                    wt_ps = tps.tile([128, 128], F32, name="wt_ps")
                    nc.tensor.transpose(wt_ps, w_rows[:, j * 128:(j + 1) * 128], ident)
                    nc.scalar.copy(out=wlhsT[j][:, ot, :], in_=wt_ps)

            xT = [xt_pool.tile([128, N], F32R, name=f"xT{j}") for j in range(2)]
            for mt in range(32):
                x_rows = tsb.tile([128, 256], F32, name="x_rows")
                nc.gpsimd.dma_start(out=x_rows, in_=x_d[mt * 128:(mt + 1) * 128, :])
                xt_ps = tps.tile([128, 256], F32, name="xt_ps")
                for j in range(2):
                    nc.tensor.transpose(
                        xt_ps[:, j * 128:(j + 1) * 128],
                        x_rows[:, j * 128:(j + 1) * 128], ident)
                for j in range(2):
                    nc.vector.tensor_copy(
                        out=xT[j][:, mt * 128:(mt + 1) * 128],
                        in_=xt_ps[:, j * 128:(j + 1) * 128])

        # ---- P2: qkv projection (f32r) -> bf16 qT/kT/vT ----
        with tc.tile_pool(name="qkv_ps", bufs=4, space="PSUM") as qps:
            for ot in range(6):
                for ch in range(NCH):
                    acc = qps.tile([128, MCH], F32, name="acc")
                    for j in range(2):
                        nc.tensor.matmul(
                            acc, wlhsT[j][:, ot, :],
                            xT[j][:, ch * MCH:(ch + 1) * MCH],
                            start=(j == 0), stop=(j == 1))
                    dst_j = ot % 2
                    if ot < 2:
                        dst = qT[dst_j][:, ch * MCH:(ch + 1) * MCH]
                    elif ot < 4:
                        dst = kT[dst_j][:, PAD + ch * MCH:PAD + (ch + 1) * MCH]
                    else:
                        dst = vT[dst_j][:, PAD + ch * MCH:PAD + (ch + 1) * MCH]
                    if ot % 2 == 0:
                        nc.scalar.copy(out=dst, in_=acc)
                    else:
                        nc.vector.tensor_copy(out=dst, in_=acc)
        xt_pool.release()

        # ---- P3: scores + softmax (m on partitions), then transpose back ----
        attn_pool = ctx.enter_context(tc.tile_pool(name="attn_sb", bufs=1))
        attn72 = attn_pool.tile([72, N], BF16)  # normalized attn weights
        r72 = attn_pool.tile([8, N], F32)       # per-head softmax reciprocal

        with tc.tile_pool(name="sc_sb", bufs=6) as ssb, \
             tc.tile_pool(name="sm_sb", bufs=8) as smb, \
             tc.tile_pool(name="st_ps", bufs=4, space="PSUM") as sps, \
             tc.tile_pool(name="at_ps", bufs=2, space="PSUM") as aps, \
             tc.tile_pool(name="rt_ps", bufs=1, space="PSUM") as rps, \
             tc.tile_pool(name="rb72_ps", bufs=1, space="PSUM") as rbps3:
            for ch in range(NCH):
                s_t = [sps.tile([128, 72], F32, name="s_t") for _ in range(4)]
                for kk in range(K2):
                    dl = DELTAS[kk]
                    for j in range(2):
                        t_t = ssb.tile([128, MCH], BF16, name="t_t")
                        nc.vector.tensor_mul(
                            t_t, qT[j][:, ch * MCH:(ch + 1) * MCH],
                            kT[j][:, PAD + ch * MCH + dl:PAD + (ch + 1) * MCH + dl])
                        for sub in range(4):
                            out_ap = s_t[sub].rearrange(
                                "p (h k) -> p h k", k=9)[:, 4 * j:4 * j + 4, kk]
                            nc.tensor.matmul(
                                out_ap, t_t[:, sub * 128:(sub + 1) * 128],
                                ones_s, start=True, stop=True)
                at_ps = aps.tile([72, 4, 128], BF16, name="at_ps")
                rt_ps = rps.tile([8, 4, 128], F32, name="rt_ps")
                for sub in range(4):
                    ms = ch * 4 + sub
                    e_t = smb.tile([128, 72], BF16, name="e_t")
                    nc.scalar.activation(
                        e_t, s_t[sub], mybir.ActivationFunctionType.Exp,
                        scale=float(SCALE))
                    em_t = smb.tile([128, 72], BF16, name="em_t")
                    nc.vector.tensor_mul(em_t, e_t, maskT[:, ms, :])
                    den = smb.tile([128, 8], F32, name="den")
                    nc.vector.reduce_sum(
                        den, em_t.rearrange("p (h k) -> p h k", k=9),
                        axis=mybir.AxisListType.X)
                    nc.vector.tensor_scalar_add(
                        out=den, in0=den, scalar1=cntT[:, ms:ms + 1])
                    rr = smb.tile([128, 8], F32, name="rr")
                    nc.vector.reciprocal(rr, den)
                    nc.tensor.transpose(at_ps[:, sub, :], em_t, identb)
                    nc.tensor.transpose(rt_ps[:, sub, :], rr, ident)
                nc.scalar.copy(
                    out=attn72[:, ch * MCH:(ch + 1) * MCH],
                    in_=at_ps.rearrange("p s q -> p (s q)"))
                nc.scalar.copy(
                    out=r72[:, ch * MCH:(ch + 1) * MCH],
                    in_=rt_ps.rearrange("p s q -> p (s q)"))
                rb72 = rbps3.tile([72, MCH], F32, name="rb72")
                nc.tensor.matmul(rb72, b9t, r72[:, ch * MCH:(ch + 1) * MCH],
                                 start=True, stop=True)
                nc.vector.tensor_mul(
                    attn72[:, ch * MCH:(ch + 1) * MCH],
                    attn72[:, ch * MCH:(ch + 1) * MCH], rb72)

        # ---- P4: weighted sum of v, transpose back, store ----
        with tc.tile_pool(name="av_sb", bufs=6) as asb, \
             tc.tile_pool(name="o_sb", bufs=4) as osb, \
             tc.tile_pool(name="bc_ps", bufs=3, space="PSUM") as bps, \
             tc.tile_pool(name="out_ps", bufs=2, space="PSUM") as ops, \
             tc.tile_pool(name="bt_ps", bufs=2, space="PSUM") as btps:
            out_view = out_d[:, :].rearrange(
                "(a t p) (j c) -> a p t j c", t=4, p=128, j=2)
            for ch in range(NCH):
                a_sl = attn72[:, ch * MCH:(ch + 1) * MCH]
                for j in range(2):
                    o_ps = ops.tile([128, MCH], F32, name="o_ps")
                    for kk in range(K2):
                        dl = DELTAS[kk]
                        bc_ps = bps.tile([128, MCH], F32, name="bc_ps")
                        nc.tensor.matmul(bc_ps, bkk[:, j, kk, :], a_sl,
                                         start=True, stop=True)
                        bc_sb = asb.tile([128, MCH], BF16, name="bc_sb")
                        nc.scalar.copy(out=bc_sb, in_=bc_ps)
                        t2 = asb.tile([128, MCH], BF16, name="t2")
                        nc.vector.tensor_mul(
                            t2, bc_sb,
                            vT[j][:, PAD + ch * MCH + dl:PAD + (ch + 1) * MCH + dl])
                        nc.tensor.matmul(o_ps, identb, t2,
                                         start=(kk == 0), stop=(kk == K2 - 1))
                    o_norm = osb.tile([128, MCH], F32, name="o_norm")
                    nc.vector.tensor_copy(out=o_norm, in_=o_ps)
                    bt_ps = btps.tile([128, 4, 128], F32, name="bt_ps")
                    for tt in range(4):
                        nc.tensor.transpose(
                            bt_ps[:, tt, :], o_norm[:, tt * 128:(tt + 1) * 128],
                            ident)
                    o_fin = osb.tile([128, 4, 128], F32, name="o_fin")
                    nc.scalar.copy(out=o_fin, in_=bt_ps)
                    nc.sync.dma_start(out=out_view[ch, :, :, j, :], in_=o_fin)
    nc.compile()
    return nc


_NC_CACHE = None


def kernel(x: np.ndarray, W_qkv: np.ndarray) -> np.ndarray:
    global _NC_CACHE
    if _NC_CACHE is None:
        _NC_CACHE = build_nc()
    nc = _NC_CACHE

    x = np.ascontiguousarray(x, dtype=np.float32)
    W_qkv = np.ascontiguousarray(W_qkv, dtype=np.float32)
    ident, identb, ones_s, bkk, b9, maskT, cntT = _host_consts()
    consts = {
        "w": W_qkv, "ident": ident, "identb": identb, "ones_s": ones_s,
        "bkk": bkk, "b9": b9, "maskT": maskT, "cntT": cntT,
    }
    in_maps = [
        {"x": x[b].reshape(N, C).copy(), **consts} for b in range(B)
    ]
    res = run_bass_kernel_spmd(nc, in_maps, list(range(B)))
    out = np.stack([res.results[b]["out"].reshape(H, W, C) for b in range(B)])
    return out


if __name__ == "__main__":
    rng = np.random.default_rng(0)
    x = rng.standard_normal((B, H, W, C), dtype=np.float32)
    wq = (rng.standard_normal((3 * C, C), dtype=np.float32) * 0.02).astype(np.float32)
    out = kernel(x, wq)
    print("out", out.shape, out.dtype, float(np.abs(out).mean()))



# revision 5
# speedup vs baseline: 1.6870x; 1.6870x over previous
"""Trainium2 Bass kernel v2 for dilated local attention (nn_DilateAttention).

Data-parallel over batch: 1 image per core. On-chip layout is channel-major
[c, m]. k/v live on a row-padded grid (row width 70 = 64 + 2*3 zeros) so every
dilated neighbor read lands either on real data or an exact zero, reproducing
the reference's zero-padding semantics (invalid slots score 0 -> exp(0)=1 in
the softmax denominator) with no masks.

Engine plan (CoreSim cost model driven):
  PE   : f32r projection, score-reduce (tiny-output matmuls), attn transpose,
         attn broadcast (72->128 via 0/1 stationary), transposed kk-accumulation
         (t2 stationary vs identity, output lands row-major)
  DVE  : q*k products (bf16 2x), softmax pieces, some AV muls
  ACT  : exp, PSUM evacuations
  Pool : AV muls (reads PSUM at full rate), evacuations, memsets
fp8 is used only for exactly-representable 0/1 constants (identity permutation
for transposes and accumulation), which the cost model rates at 1 cycle/row.
"""

import sys

sys.path.insert(0, "/opt/trn_rl_repo")

import numpy as np
import ml_dtypes
from contextlib import ExitStack

import concourse.bass as bass
import concourse.bacc as bacc
import concourse.tile as tile
from concourse import mybir
from concourse.bass_utils import run_bass_kernel_spmd

B, H, W, C = 8, 64, 64, 256
NH, DPH, K2 = 8, 32, 9
N = H * W            # 4096
WP = W + 6           # 70: padded row width
NP = H * WP          # 4480
PADT = 216           # top/bottom zero pad (need >= 3*70+3 = 213)
NPT = PADT + NP + PADT  # 4912
SCALE = DPH ** -0.5
ESCALE = SCALE

F32 = mybir.dt.float32
F32R = mybir.dt.float32r
BF16 = mybir.dt.bfloat16
FP8 = mybir.dt.float8e4
NPBF16 = ml_dtypes.bfloat16
NPFP8 = np.dtype(mybir.dt.np(FP8))

DR = mybir.MatmulPerfMode.DoubleRow

# delta' on the padded-row grid for kk = 3*i + jj, i/jj in {0,1,2}
DELTAS = [WP * (3 * i - 3) + (3 * jj - 3) for i in range(3) for jj in range(3)]

MCH_P = 512    # projection m-chunk
MCH_S = 512    # score/softmax m-chunk (8 image rows)
MCH_A = 512    # AV m-chunk


def _host_consts():
    identb = np.eye(128, dtype=NPBF16)
    identf = np.eye(128, dtype=np.float32)
    # score reduce moving operand: ones4[p, n] = 1 iff p//32 == n
    ones4 = np.zeros((128, 4), NPBF16)
    for p in range(128):
        ones4[p, p // 32] = 1.0
    # broadcast stationary: S[f, c] = 1 iff f == (4j + c//32)*9 + kk
    sbb = np.zeros((72, 2, K2, 128), NPBF16)
    for j in range(2):
        for kk in range(K2):
            for c in range(128):
                sbb[(4 * j + c // 32) * 9 + kk, j, kk, c] = 1.0
    sbb = sbb.reshape(72, 2 * K2 * 128)
    return identb, identf, ones4, sbb


def build_nc() -> bass.Bass:
    nc = bacc.Bacc()
    x_d = nc.declare_dram_parameter("x", [N, C], F32, isOutput=False)
    w_d = nc.declare_dram_parameter("w", [3 * C, C], F32, isOutput=False)
    identb_d = nc.declare_dram_parameter("identb", [128, 128], BF16, isOutput=False)
    identf_d = nc.declare_dram_parameter("identf", [128, 128], F32, isOutput=False)
    ones4_d = nc.declare_dram_parameter("ones4", [128, 4], BF16, isOutput=False)
    sbb_d = nc.declare_dram_parameter("sbb", [72, 2 * K2 * 128], BF16, isOutput=False)
    out_d = nc.declare_dram_parameter("out", [N, C], F32, isOutput=True)

    with tile.TileContext(nc) as tc, ExitStack() as ctx:
        singles = ctx.enter_context(tc.tile_pool(name="singles", bufs=1))
        big = ctx.enter_context(tc.tile_pool(name="big", bufs=1))

        identb = singles.tile([128, 128], BF16)
        nc.scalar.dma_start(out=identb, in_=identb_d[:, :])
        identf = singles.tile([128, 128], F32)
        nc.scalar.dma_start(out=identf, in_=identf_d[:, :])
        ones4 = singles.tile([128, 4], BF16)
        nc.scalar.dma_start(out=ones4, in_=ones4_d[:, :])
        sbb = singles.tile([72, 2, K2, 128], BF16)
        nc.scalar.dma_start(
            out=sbb, in_=sbb_d[:, :].rearrange("p (j k c) -> p j k c", j=2, k=K2))

        # persistent big tensors
        xTr = big.tile([128, 2, N], F32R)         # x^T, f32r
        wlhsT = singles.tile([128, 6, 2, 128], F32R)
        qT = big.tile([128, 2, N], BF16)
        kTp = big.tile([128, 2, NPT], BF16)
        vTp = big.tile([128, 2, NPT], BF16)
        e72 = big.tile([72, N], BF16)             # normalized attn, f=(h,kk) major

        # zero borders of kTp/vTp: head, tail, and the 6-wide per-row gaps
        for t in (kTp, vTp):
            nc.gpsimd.memset(t[:, :, 0:PADT + 3], 0.0)
            nc.gpsimd.memset(t[:, :, PADT + (H - 1) * WP + 3 + W:], 0.0)
            gap = t[:, :, PADT + 3 + W:PADT + 3 + W + (H - 1) * WP].rearrange(
                "p j (r q) -> p j r q", q=WP)[:, :, :, 0:6]
            nc.gpsimd.memset(gap, 0.0)

        # ---- P1: load w + x, transpose to channel-major, cast to fp8 ----
        x_view = x_d[:, :].rearrange("(t b p) c -> t p b c", b=4, p=128)
        w6 = singles.tile([128, 6, 256], F32)

        with tc.tile_pool(name="xr", bufs=3) as xrp, \
             tc.tile_pool(name="tp_ps", bufs=4, space="PSUM") as tpps:
            # x tile 0 first so transposes start ASAP, then w
            xr0 = xrp.tile([128, 4, 256], F32, name="xr")
            nc.sync.dma_start(out=xr0, in_=x_view[0])
            nc.sync.dma_start(
                out=w6, in_=w_d[:, :].rearrange("(t p) c -> p t c", p=128))
            # w: transpose to [c, o] tiles (f32r)
            for ot in range(6):
                wt_ps = tpps.tile([128, 2, 128], F32, name="wt_ps")
                for j in range(2):
                    nc.tensor.transpose(
                        wt_ps[:, j, :], w6[:, ot, j * 128:(j + 1) * 128], identf)
                nc.scalar.copy(out=wlhsT[:, ot, :, :], in_=wt_ps)
            # x: 8 DMA tiles of 4 row-blocks; transpose f32r; evacuate
            for ti in range(8):
                if ti == 0:
                    xr = xr0
                else:
                    xr = xrp.tile([128, 4, 256], F32, name="xr")
                    eng = nc.sync if ti % 2 == 0 else nc.scalar
                    eng.dma_start(out=xr, in_=x_view[ti])
                xt_ps = tpps.tile([128, 4, 128], F32, name="xt_ps")
                for j in range(2):
                    for b in range(4):
                        nc.tensor.transpose(
                            xt_ps[:, b, :], xr[:, b, j * 128:(j + 1) * 128], identf)
                    dst = xTr[:, j, ti * 512:(ti + 1) * 512]
                    src = xt_ps.rearrange("p b q -> p (b q)")
                    if j == 0:
                        nc.scalar.copy(out=dst, in_=src)
                    else:
                        nc.vector.tensor_copy(out=dst, in_=src)

        # ---- P2..P4 interleaved over chunks ----
        NCH_P = N // MCH_P
        with tc.tile_pool(name="proj_ps", bufs=2, space="PSUM") as pps, \
             tc.tile_pool(name="s_ps", bufs=1, space="PSUM") as sps, \
             tc.tile_pool(name="at_ps", bufs=2, space="PSUM") as atps, \
             tc.tile_pool(name="bc_ps", bufs=2, space="PSUM") as bcps, \
             tc.tile_pool(name="oT_ps", bufs=1, space="PSUM") as otps, \
             tc.tile_pool(name="t9p", bufs=2) as t9p, \
             tc.tile_pool(name="smx", bufs=4) as smx, \
             tc.tile_pool(name="t2p", bufs=2) as t2p, \
             tc.tile_pool(name="outp", bufs=2) as outp:

            # ---------- projection ----------
            def proj_chunk(ch):
                r0 = ch * MCH_P // W          # 8 image rows per chunk
                for ot in range(6):
                    acc = pps.tile([128, MCH_P], F32, name="acc")
                    for i in range(2):
                        nc.tensor.matmul(
                            acc,
                            wlhsT[:, ot, i, :],
                            xTr[:, i, ch * MCH_P:(ch + 1) * MCH_P],
                            start=(i == 0), stop=(i == 1))
                    j = ot % 2
                    if ot < 2:
                        dst = qT[:, j, ch * MCH_P:(ch + 1) * MCH_P]
                        src = acc
                    else:
                        t = kTp if ot < 4 else vTp
                        dst = t[:, j, PADT + r0 * WP + 3:
                                PADT + (r0 + 8) * WP + 3].rearrange(
                            "p (r q) -> p r q", q=WP)[:, :, 0:W]
                        src = acc.rearrange("p (r q) -> p r q", q=W)
                    if ot in (0, 5):
                        nc.vector.tensor_copy(out=dst, in_=src)
                    else:
                        nc.scalar.copy(out=dst, in_=src)

            # ---------- scores + softmax for one MCH_S chunk ----------
            def scores_chunk(ch):
                r0 = ch * MCH_S // W          # 8 image rows
                base = PADT + r0 * WP + 3
                t9 = [t9p.tile([128, K2, MCH_S], BF16, name=f"t9_{j}")
                      for j in range(2)]
                for j in range(2):
                    qs = qT[:, j, ch * MCH_S:(ch + 1) * MCH_S].rearrange(
                        "p (r q) -> p r q", q=W)
                    dve_kk = (1, 3, 5, 7) if j == 0 else (3, 5)
                    for kk in range(K2):
                        dl = DELTAS[kk]
                        ks = kTp[:, j, base + dl:base + dl + 8 * WP].rearrange(
                            "p (r q) -> p r q", q=WP)[:, :, 0:W]
                        eng = nc.vector if kk in dve_kk else nc.gpsimd
                        eng.tensor_mul(
                            t9[j][:, kk, :].rearrange("p (r q) -> p r q", q=W),
                            qs, ks)
                s4 = sps.tile([128, 4, 72], F32, name="s4")
                for j in range(2):
                    for kk in range(K2):
                        for sub in range(4):
                            out_ap = s4.rearrange(
                                "p s (h k) -> p s h k", k=K2)[:, sub, 4 * j:4 * j + 4, kk]
                            nc.tensor.matmul(
                                out_ap, t9[j][:, kk, sub * 128:(sub + 1) * 128],
                                ones4, start=True, stop=True)
                at_ps = atps.tile([72, 4, 128], BF16, name="at_ps")
                for sub in range(4):
                    em = smx.tile([128, 72], BF16, name="em")
                    nc.scalar.activation(
                        em, s4[:, sub, :], mybir.ActivationFunctionType.Exp,
                        scale=float(ESCALE))
                    den = smx.tile([128, 8], F32, name="den")
                    nc.vector.reduce_sum(
                        den, em.rearrange("p (h k) -> p h k", k=K2),
                        axis=mybir.AxisListType.X)
                    r8 = smx.tile([128, 8], F32, name="r8")
                    nc.vector.reciprocal(r8, den)
                    em_n = smx.tile([128, 72], BF16, name="em_n")
                    nc.vector.tensor_mul(
                        em_n,
                        em.rearrange("p (h k) -> p h k", k=K2),
                        r8[:, :, None].broadcast_to((128, 8, K2)))
                    nc.tensor.transpose(at_ps[:, sub, :], em_n, identb)
                nc.scalar.copy(
                    out=e72[:, ch * MCH_S:(ch + 1) * MCH_S],
                    in_=at_ps.rearrange("p s q -> p (s q)"))

            # ---------- AV for one MCH_A chunk ----------
            t2_live = {}

            def av_bcmul(ch, j, lo=0, hi=MCH_A, tag=""):
                r0 = (ch * MCH_A + lo) // W
                nrow = (hi - lo) // W
                base = PADT + r0 * WP + 3
                e_sl = e72[:, ch * MCH_A + lo:ch * MCH_A + hi]
                t2 = t2p.tile([128, K2, hi - lo], BF16, name=f"t2_{j}")
                t2_live[(ch, j, lo)] = t2
                for kk in range(K2):
                    dl = DELTAS[kk]
                    bc = bcps.tile([128, hi - lo], F32, name="bc")
                    nc.tensor.matmul(
                        bc, sbb[:, j, kk, :], e_sl, start=True, stop=True)
                    vs = vTp[:, j, base + dl:base + dl + nrow * WP].rearrange(
                        "p (r q) -> p r q", q=WP)[:, :, 0:W]
                    dst = t2[:, kk, :].rearrange("p (r q) -> p r q", q=W)
                    bcr = bc.rearrange("p (r q) -> p r q", q=W)
                    if kk in (2, 4, 6, 8):
                        # ACT evacuates, Pool multiplies from SBUF
                        bc_sb = outp.tile([128, hi - lo], BF16, name="bc_sb")
                        nc.scalar.copy(out=bc_sb, in_=bc)
                        nc.gpsimd.tensor_mul(dst, bc_sb.rearrange(
                            "p (r q) -> p r q", q=W), vs)
                    else:
                        nc.vector.tensor_mul(dst, bcr, vs)

            def av_accum(ch, j, lo=0, hi=MCH_A, tag=""):
                t2 = t2_live.pop((ch, j, lo))
                nsub = (hi - lo) // 128
                # transposed accumulation: out lands row-major in PSUM
                oT_ps = otps.tile([128, nsub, 128], F32, name="oT_ps")
                for sub in range(nsub):
                    for kk in range(K2):
                        nc.tensor.matmul(
                            oT_ps[:, sub, :],
                            t2[:, kk, sub * 128:(sub + 1) * 128], identb,
                            start=(kk == 0), stop=(kk == K2 - 1))
                o_fin = outp.tile([128, nsub, 128], F32, name="o_fin")
                if ch % 4 == 3:
                    nc.vector.tensor_copy(out=o_fin, in_=oT_ps)
                else:
                    nc.scalar.copy(out=o_fin, in_=oT_ps)
                out_view = out_d[:, :].rearrange(
                    "(a t p) (j c) -> a p t j c", t=nsub, p=128, j=2)
                a0 = (ch * MCH_A + lo) // (nsub * 128)
                nc.sync.dma_start(out=out_view[a0, :, :, j, :], in_=o_fin)

            # ---------- interleaved schedule ----------
            # proj runs 2 chunks (of MCH_P) ahead; scores 1 MCH_S chunk ahead of AV
            NCH_S = N // MCH_S
            for pch in range(2):
                proj_chunk(pch)
            scores_chunk(0)
            for ch in range(NCH_S - 1):
                pch = ch + 2
                if pch < NCH_P:
                    proj_chunk(pch)
                av_bcmul(ch, 0)
                av_bcmul(ch, 1)
                if ch + 1 < NCH_S:
                    scores_chunk(ch + 1)
                if ch > 0:
                    av_accum(ch - 1, 0)
                    av_accum(ch - 1, 1)
            ch = NCH_S - 1
            av_bcmul(ch, 0)
            av_bcmul(ch, 1)
            av_accum(ch - 1, 0)
            av_accum(ch - 1, 1)
            av_accum(ch, 0)
            av_accum(ch, 1)
    nc.compile()
    return nc


_NC_CACHE = None


def _in_map(x_img: np.ndarray, W_qkv: np.ndarray) -> dict:
    identb, identf, ones4, sbb = _host_consts()
    return {
        "x": np.ascontiguousarray(x_img.reshape(N, C), np.float32),
        "w": np.ascontiguousarray(W_qkv, np.float32),
        "identb": identb, "identf": identf, "ones4": ones4, "sbb": sbb,
    }


def kernel(x: np.ndarray, W_qkv: np.ndarray) -> np.ndarray:
    global _NC_CACHE
    if _NC_CACHE is None:
        _NC_CACHE = build_nc()
    nc = _NC_CACHE
    in_maps = [_in_map(x[b], W_qkv) for b in range(B)]
    res = run_bass_kernel_spmd(nc, in_maps, list(range(B)))
    out = np.stack([res.results[b]["out"].reshape(H, W, C) for b in range(B)])
    return out


if __name__ == "__main__":
    rng = np.random.default_rng(0)
    x = rng.standard_normal((B, H, W, C), dtype=np.float32)
    wq = (rng.standard_normal((3 * C, C), dtype=np.float32) * 0.02).astype(np.float32)
    out = kernel(x, wq)
    print("out", out.shape, out.dtype, float(np.abs(out).mean()))


# revision 21
# speedup vs baseline: 1.9223x; 1.1395x over previous
"""Trainium2 Bass kernel v2 for dilated local attention (nn_DilateAttention).

Data-parallel over batch: 1 image per core. On-chip layout is channel-major
[c, m]. k/v live on a row-padded grid (row width 70 = 64 + 2*3 zeros) so every
dilated neighbor read lands either on real data or an exact zero, reproducing
the reference's zero-padding semantics (invalid slots score 0 -> exp(0)=1 in
the softmax denominator) with no masks.

Engine plan (CoreSim cost model driven):
  PE   : f32r projection, score-reduce (tiny-output matmuls), attn transpose,
         attn broadcast (72->128 via 0/1 stationary), transposed kk-accumulation
         (t2 stationary vs identity, output lands row-major)
  DVE  : q*k products (bf16 2x), softmax pieces, some AV muls
  ACT  : exp, PSUM evacuations
  Pool : AV muls (reads PSUM at full rate), evacuations, memsets
fp8 is used only for exactly-representable 0/1 constants (identity permutation
for transposes and accumulation), which the cost model rates at 1 cycle/row.
"""

import sys

sys.path.insert(0, "/opt/trn_rl_repo")

import numpy as np
import ml_dtypes
from contextlib import ExitStack

import concourse.bass as bass
import concourse.bacc as bacc
import concourse.tile as tile
from concourse import mybir
from concourse.bass_utils import run_bass_kernel_spmd

B, H, W, C = 8, 64, 64, 256
NH, DPH, K2 = 8, 32, 9
N = H * W            # 4096
WP = W + 6           # 70: padded row width
NP = H * WP          # 4480
PADT = 216           # top/bottom zero pad (need >= 3*70+3 = 213)
NPT = PADT + NP + PADT  # 4912
SCALE = DPH ** -0.5
ESCALE = SCALE

F32 = mybir.dt.float32
F32R = mybir.dt.float32r
BF16 = mybir.dt.bfloat16
FP8 = mybir.dt.float8e4
NPBF16 = ml_dtypes.bfloat16
NPFP8 = np.dtype(mybir.dt.np(FP8))

DR = mybir.MatmulPerfMode.DoubleRow

# delta' on the padded-row grid for kk = 3*i + jj, i/jj in {0,1,2}
DELTAS = [WP * (3 * i - 3) + (3 * jj - 3) for i in range(3) for jj in range(3)]

MCH_P = 512    # projection m-chunk
MCH_S = 512    # score/softmax m-chunk (8 image rows)
MCH_A = 512    # AV m-chunk


def _host_consts():
    identb = np.eye(128, dtype=NPBF16)
    identf = np.eye(128, dtype=np.float32)
    # score reduce moving operand: ones4[p, n] = 1 iff p//32 == n
    ones4 = np.zeros((128, 4), NPBF16)
    for p in range(128):
        ones4[p, p // 32] = 1.0
    # broadcast stationary: S[f, c] = 1 iff f == (4j + c//32)*9 + kk
    sbb = np.zeros((72, 2, K2, 128), NPBF16)
    for j in range(2):
        for kk in range(K2):
            for c in range(128):
                sbb[(4 * j + c // 32) * 9 + kk, j, kk, c] = 1.0
    sbb = sbb.reshape(72, 2 * K2 * 128)
    return identb, identf, ones4, sbb


def build_nc() -> bass.Bass:
    nc = bacc.Bacc()
    x_d = nc.declare_dram_parameter("x", [N, C], F32, isOutput=False)
    w_d = nc.declare_dram_parameter("w", [3 * C, C], F32, isOutput=False)
    identb_d = nc.declare_dram_parameter("identb", [128, 128], BF16, isOutput=False)
    identf_d = nc.declare_dram_parameter("identf", [128, 128], F32, isOutput=False)
    ones4_d = nc.declare_dram_parameter("ones4", [128, 4], BF16, isOutput=False)
    sbb_d = nc.declare_dram_parameter("sbb", [72, 2 * K2 * 128], BF16, isOutput=False)
    out_d = nc.declare_dram_parameter("out", [N, C], F32, isOutput=True)

    with tile.TileContext(nc) as tc, ExitStack() as ctx:
        singles = ctx.enter_context(tc.tile_pool(name="singles", bufs=1))
        big = ctx.enter_context(tc.tile_pool(name="big", bufs=1))

        identb = singles.tile([128, 128], BF16)
        nc.gpsimd.dma_start(out=identb, in_=identb_d[:, :])
        identf = singles.tile([128, 128], F32)
        nc.gpsimd.dma_start(out=identf, in_=identf_d[:, :])
        ones4 = singles.tile([128, 4], BF16)
        sbb = singles.tile([72, 2, K2, 128], BF16)

        # persistent big tensors
        xTr = big.tile([128, 2, N], F32R)         # x^T, f32r
        wlhsT = singles.tile([128, 6, 2, 128], F32R)
        qT = big.tile([128, 2, N], BF16)
        kTp = big.tile([128, 2, NPT], BF16)
        vTp = big.tile([128, 2, NPT], BF16)
        e72 = big.tile([72, N], BF16)             # normalized attn, f=(h,kk) major

        # zero borders of kTp/vTp: head, tail, and the 6-wide per-row gaps
        for t in (kTp, vTp):
            nc.gpsimd.memset(t[:, :, 0:PADT + 3], 0.0)
            nc.gpsimd.memset(t[:, :, PADT + (H - 1) * WP + 3 + W:], 0.0)
            gap = t[:, :, PADT + 3 + W:PADT + 3 + W + (H - 1) * WP].rearrange(
                "p j (r q) -> p j r q", q=WP)[:, :, :, 0:6]
            nc.gpsimd.memset(gap, 0.0)

        # ---- P1: load w + x, transpose to channel-major, cast to fp8 ----
        x_view = x_d[:, :].rearrange("(t b p) c -> t p b c", b=4, p=128)
        w6 = singles.tile([128, 6, 256], F32)

        with tc.tile_pool(name="xr", bufs=3) as xrp, \
             tc.tile_pool(name="tp_ps", bufs=4, space="PSUM") as tpps:
            # x tile 0 first so transposes start ASAP, then w
            xr0 = xrp.tile([128, 4, 256], F32, name="xr")
            nc.sync.dma_start(out=xr0, in_=x_view[0])
            nc.sync.dma_start(
                out=w6, in_=w_d[:, :].rearrange("(t p) c -> p t c", p=128))
            # w: transpose to [c, o] tiles
            for ot in range(6):
                wt_ps = tpps.tile([128, 2, 128], F32, name="wt_ps")
                for j in range(2):
                    nc.tensor.transpose(
                        wt_ps[:, j, :], w6[:, ot, j * 128:(j + 1) * 128], identf)
                nc.scalar.copy(out=wlhsT[:, ot, :, :], in_=wt_ps)
            # x: 8 DMA tiles of 4 row-blocks; transpose; evacuate
            for ti in range(8):
                if ti == 0:
                    xr = xr0
                else:
                    xr = xrp.tile([128, 4, 256], F32, name="xr")
                    eng = nc.sync if ti % 2 == 0 else nc.gpsimd
                    eng.dma_start(out=xr, in_=x_view[ti])
                xt_ps = tpps.tile([128, 4, 128], F32, name="xt_ps")
                for j in range(2):
                    for b in range(4):
                        nc.tensor.transpose(
                            xt_ps[:, b, :], xr[:, b, j * 128:(j + 1) * 128], identf)
                    dst = xTr[:, j, ti * 512:(ti + 1) * 512]
                    src = xt_ps.rearrange("p b q -> p (b q)")
                    if j == 0:
                        nc.scalar.copy(out=dst, in_=src)
                    else:
                        nc.vector.tensor_copy(out=dst, in_=src)
                if ti == 1:
                    nc.gpsimd.dma_start(out=ones4, in_=ones4_d[:, :])
                    nc.gpsimd.dma_start(
                        out=sbb,
                        in_=sbb_d[:, :].rearrange("p (j k c) -> p j k c", j=2, k=K2))

        # ---- P2..P4 interleaved over chunks ----
        NCH_P = N // MCH_P
        with tc.tile_pool(name="proj_ps", bufs=2, space="PSUM") as pps, \
             tc.tile_pool(name="s_ps", bufs=1, space="PSUM") as sps, \
             tc.tile_pool(name="at_ps", bufs=1, space="PSUM") as atps, \
             tc.tile_pool(name="bc_ps", bufs=3, space="PSUM") as bcps, \
             tc.tile_pool(name="oT_ps", bufs=1, space="PSUM") as otps, \
             tc.tile_pool(name="t9p", bufs=2) as t9p, \
             tc.tile_pool(name="smx", bufs=4) as smx, \
             tc.tile_pool(name="t2p", bufs=2) as t2p, \
             tc.tile_pool(name="outp", bufs=2) as outp:

            # ---------- projection ----------
            def proj_chunk(ch):
                r0 = ch * MCH_P // W          # 8 image rows per chunk
                for ot in range(6):
                    acc = pps.tile([128, MCH_P], F32, name="acc")
                    for i in range(2):
                        nc.tensor.matmul(
                            acc,
                            wlhsT[:, ot, i, :],
                            xTr[:, i, ch * MCH_P:(ch + 1) * MCH_P],
                            start=(i == 0), stop=(i == 1))
                    j = ot % 2
                    if ot < 2:
                        dst = qT[:, j, ch * MCH_P:(ch + 1) * MCH_P]
                        src = acc
                    else:
                        t = kTp if ot < 4 else vTp
                        dst = t[:, j, PADT + r0 * WP + 3:
                                PADT + (r0 + 8) * WP + 3].rearrange(
                            "p (r q) -> p r q", q=WP)[:, :, 0:W]
                        src = acc.rearrange("p (r q) -> p r q", q=W)
                    if ot in (0, 5):
                        nc.vector.tensor_copy(out=dst, in_=src)
                    else:
                        nc.scalar.copy(out=dst, in_=src)

            # ---------- scores + softmax for one MCH_S chunk ----------
            def scores_chunk(ch):
                r0 = ch * MCH_S // W          # 8 image rows
                base = PADT + r0 * WP + 3
                t9 = [t9p.tile([128, K2, MCH_S], BF16, name=f"t9_{j}")
                      for j in range(2)]
                for j in range(2):
                    qs = qT[:, j, ch * MCH_S:(ch + 1) * MCH_S].rearrange(
                        "p (r q) -> p r q", q=W)
                    dve_kk = (1, 3, 5, 7) if j == 0 else (3,)
                    for kk in range(K2):
                        dl = DELTAS[kk]
                        ks = kTp[:, j, base + dl:base + dl + 8 * WP].rearrange(
                            "p (r q) -> p r q", q=WP)[:, :, 0:W]
                        eng = nc.vector if kk in dve_kk else nc.gpsimd
                        eng.tensor_mul(
                            t9[j][:, kk, :].rearrange("p (r q) -> p r q", q=W),
                            qs, ks)
                s4 = sps.tile([128, 4, 72], F32, name="s4")
                for j in range(2):
                    for kk in range(K2):
                        for sub in range(4):
                            out_ap = s4.rearrange(
                                "p s (h k) -> p s h k", k=K2)[:, sub, 4 * j:4 * j + 4, kk]
                            nc.tensor.matmul(
                                out_ap, t9[j][:, kk, sub * 128:(sub + 1) * 128],
                                ones4, start=True, stop=True)
                at_ps = atps.tile([72, 4, 128], BF16, name="at_ps")
                em = smx.tile([128, 4, 72], BF16, name="em")
                nc.scalar.activation(
                    em, s4, mybir.ActivationFunctionType.Exp,
                    scale=float(ESCALE))
                den = smx.tile([128, 4, 8], F32, name="den")
                nc.vector.reduce_sum(
                    den, em.rearrange("p s (h k) -> p s h k", k=K2),
                    axis=mybir.AxisListType.X)
                r8 = smx.tile([128, 4, 8], F32, name="r8")
                nc.vector.reciprocal(r8, den)
                em_n = smx.tile([128, 4, 72], BF16, name="em_n")
                nc.vector.tensor_mul(
                    em_n,
                    em.rearrange("p s (h k) -> p s h k", k=K2),
                    r8[:, :, :, None].broadcast_to((128, 4, 8, K2)))
                for sub in range(4):
                    nc.tensor.transpose(at_ps[:, sub, :], em_n[:, sub, :], identb)
                nc.scalar.copy(
                    out=e72[:, ch * MCH_S:(ch + 1) * MCH_S],
                    in_=at_ps.rearrange("p s q -> p (s q)"))

            # ---------- AV for one MCH_A chunk ----------
            t2_live = {}

            def av_bcmul(ch, j, lo=0, hi=MCH_A, tag=""):
                r0 = (ch * MCH_A + lo) // W
                nrow = (hi - lo) // W
                base = PADT + r0 * WP + 3
                e_sl = e72[:, ch * MCH_A + lo:ch * MCH_A + hi]
                t2 = t2p.tile([128, K2, hi - lo], BF16, name=f"t2_{j}")
                t2_live[(ch, j, lo)] = t2
                for kk in range(K2):
                    dl = DELTAS[kk]
                    bc = bcps.tile([128, hi - lo], F32, name="bc")
                    nc.tensor.matmul(
                        bc, sbb[:, j, kk, :], e_sl, start=True, stop=True)
                    vs = vTp[:, j, base + dl:base + dl + nrow * WP].rearrange(
                        "p (r q) -> p r q", q=WP)[:, :, 0:W]
                    dst = t2[:, kk, :].rearrange("p (r q) -> p r q", q=W)
                    bcr = bc.rearrange("p (r q) -> p r q", q=W)
                    route = (0, 2, 4, 6, 8) if j == 0 else (2, 4, 6, 8)
                    if kk in route:
                        # ACT evacuates, Pool multiplies from SBUF
                        bc_sb = outp.tile([128, hi - lo], BF16, name="bc_sb")
                        nc.scalar.copy(out=bc_sb, in_=bc)
                        nc.gpsimd.tensor_mul(dst, bc_sb.rearrange(
                            "p (r q) -> p r q", q=W), vs)
                    else:
                        nc.vector.tensor_mul(dst, bcr, vs)

            def av_accum(ch, j, lo=0, hi=MCH_A, tag=""):
                t2 = t2_live.pop((ch, j, lo))
                nsub = (hi - lo) // 128
                # transposed accumulation: out lands row-major in PSUM
                oT_ps = otps.tile([128, nsub, 128], F32, name="oT_ps")
                for sub in range(nsub):
                    for kk in range(K2):
                        nc.tensor.matmul(
                            oT_ps[:, sub, :],
                            t2[:, kk, sub * 128:(sub + 1) * 128], identb,
                            start=(kk == 0), stop=(kk == K2 - 1))
                o_fin = outp.tile([128, nsub, 128], F32, name="o_fin")
                if ch % 4 == 3:
                    nc.vector.tensor_copy(out=o_fin, in_=oT_ps)
                else:
                    nc.scalar.copy(out=o_fin, in_=oT_ps)
                out_view = out_d[:, :].rearrange(
                    "(a t p) (j c) -> a p t j c", t=nsub, p=128, j=2)
                a0 = (ch * MCH_A + lo) // (nsub * 128)
                nc.sync.dma_start(out=out_view[a0, :, :, j, :], in_=o_fin)

            # ---------- interleaved schedule ----------
            # proj runs 2 chunks (of MCH_P) ahead; scores 1 MCH_S chunk ahead of AV
            NCH_S = N // MCH_S
            for pch in range(2):
                proj_chunk(pch)
            scores_chunk(0)
            for ch in range(NCH_S - 1):
                pch = ch + 2
                if pch < NCH_P:
                    proj_chunk(pch)
                av_bcmul(ch, 0)
                av_bcmul(ch, 1)
                if ch + 1 < NCH_S:
                    scores_chunk(ch + 1)
                if ch > 0:
                    av_accum(ch - 1, 0)
                    av_accum(ch - 1, 1)
            ch = NCH_S - 1
            av_bcmul(ch, 0)
            av_bcmul(ch, 1)
            av_accum(ch - 1, 0)
            av_accum(ch - 1, 1)
            av_accum(ch, 0)
            av_accum(ch, 1)
    nc.compile()
    return nc


_NC_CACHE = None


def _in_map(x_img: np.ndarray, W_qkv: np.ndarray) -> dict:
    identb, identf, ones4, sbb = _host_consts()
    return {
        "x": np.ascontiguousarray(x_img.reshape(N, C), np.float32),
        "w": np.ascontiguousarray(W_qkv, np.float32),
        "identb": identb, "identf": identf, "ones4": ones4, "sbb": sbb,
    }


def kernel(x: np.ndarray, W_qkv: np.ndarray) -> np.ndarray:
    global _NC_CACHE
    if _NC_CACHE is None:
        _NC_CACHE = build_nc()
    nc = _NC_CACHE
    in_maps = [_in_map(x[b], W_qkv) for b in range(B)]
    res = run_bass_kernel_spmd(nc, in_maps, list(range(B)))
    out = np.stack([res.results[b]["out"].reshape(H, W, C) for b in range(B)])
    return out


if __name__ == "__main__":
    rng = np.random.default_rng(0)
    x = rng.standard_normal((B, H, W, C), dtype=np.float32)
    wq = (rng.standard_normal((3 * C, C), dtype=np.float32) * 0.02).astype(np.float32)
    out = kernel(x, wq)
    print("out", out.shape, out.dtype, float(np.abs(out).mean()))
